# revision 41
# baseline (speedup 1.0000x reference)
"""Bahdanau additive attention (ragged sequence) on 8 Trainium2 NeuronCores.

Reference math (per batch b over sequence l, d=512, a=64):
    parts  = enc @ W_enc + b_attn                        (l, a)
    scores = tanh(parts + hidden @ W_hidden) . v         (l,)
    w      = softmax(scores + mask) over l               (valid: l < lens[b])
    out[b] = sum_l w[l] * enc[l, b, :]                   (512,)

Strategy (batch-parallel over 8 cores, 8 batches each; single pass over enc):
  * Host packs enc per (slot, d-chunk) into one contiguous fp16 buffer
    [128, sum_j 4*(n_l_j+16)] including 16 zero-pad columns per section, so
    each slot is ONE sync-queue DMA with ~16KB/partition contiguous rows
    (max descriptor efficiency, 1 HWDGE instruction per slot instead of 4).
  * Ragged skipping at 128-column granularity (positions l >= lens[b]
    contribute exactly 0); host sorts batches by chunk count, deals them
    across cores, compiles against the per-slot max-chunk template.
  * The PE p-state drops to 1.2 GHz on any idle gap and needs ~3us of
    continuous work to regain 2.4 GHz. So: a computed run of warmup matmuls
    keeps the PE hot until the DMA stream has enough lead that the real
    matmul stream (stage A + score) then runs with zero dependency gaps.
    Emission is software-pipelined: stage A of group g+1 is issued between
    stage A(g) and score(g) so the tanh (ACT) latency is hidden.
  * Scores group 1024 columns per PSUM tile: one tanh + one exp per group.
    Length mask (host-built 0/-30000 fp8 rows) accumulates into the score
    PSUM via an extra PE matmul only for 512-col subchunks past the slot's
    min valid length. exp runs on ACT out of PSUM to BF16 with accum_out
    giving the softmax denominator for free (exp(-30000) == 0 exactly).
  * Stage B: custom DVE op (2 elem/cycle/lane packed 16-bit) computes the
    running total of et*p per (slot, d-chunk); the settled total is read
    from the zero-padded tail. Softmax normalization happens on HOST (the
    kernel outputs unnormalized sums R and denominators S).
"""
import sys

sys.path.insert(0, "/opt/trn_rl_repo")

from contextlib import ExitStack

import ml_dtypes
import numpy as np

import concourse.bacc as bacc
import concourse.bass as bass  # noqa: F401  (kept for debugging)
import concourse.bass_isa as bass_isa
import concourse.dve_ops as dve_ops
import concourse.tile as tile
from concourse import mybir
from concourse.bass_utils import run_bass_kernel_spmd
from concourse.dve_spec import Spec, Src0, Src1, Zero, lower as dve_lower
from concourse.dve_uop import (
    AluInp,
    AluOp,
    DelayInp,
    DveOpSpec,
    InpSel,
    OutPath,
    OutSel,
    Trigger,
    UopConfig,
)
from operator import add as _op_add

F32 = mybir.dt.float32
F16 = mybir.dt.float16
BF16 = mybir.dt.bfloat16
F8 = mybir.dt.float8e5

N_CORES = 8
L, B, D, A, H = 2048, 64, 512, 64, 512
BL = B // N_CORES  # local batches per core
DC = D // 128  # d-chunks of 128 partitions
CHUNK = 128  # l-chunk width for ragged skipping
GCOL = 1024  # columns per PSUM score/parts group (2 banks)
SUB = 512  # matmul moving-operand width
NCH = L // CHUNK
PADC = 16  # zero-padded tail columns for the MUL_ACC_2X ripple readback

# ---------------------------------------------------------------------------
# MUL_ACC_2X: custom DVE op computing accum_out[p] = sum_k in0[p,k]*in1[p,k]
# in one pass at 2 elem/cycle/lane for 16-bit operands. See baseline notes:
# running totals ripple down the ALU chain; the settled total is read from
# the zero-padded tail region (col n-4 of a 16-padded stream).


def _mul_acc_2x_uops():
    lanes = [InpSel.SRC_0, InpSel.SRC_1, InpSel.SRC_0_HI, InpSel.SRC_1_HI,
             InpSel.ZERO]

    def base():
        u = UopConfig()
        for i, s in enumerate(lanes):
            u.enable_input(s, i)
        u.accum_enabled = 1
        u.datapath_config[0].enable_alu(
            AluOp.MULTIPLY, AluInp.PREV_ALU_OUT, AluInp.PREV_DELAY_0
        ).pass_through_delay(1, 2, 3)
        u.datapath_config[1].enable_alu(
            AluOp.MULTIPLY, AluInp.PREV_DELAY_1, AluInp.PREV_DELAY_2
        ).enable_delay_from_src(DelayInp.PREV_ALU_OUT, 0).pass_through_delay(3)
        u.datapath_config[2].enable_delay_from_src(
            DelayInp.PREV_ALU_OUT, 1
        ).pass_through_delay(0, 3)
        for b in range(3, 8):
            u.datapath_config[b].pass_through_delay(0, 1)
        for b in range(4, 8):
            u.datapath_config[b].pass_through_alu()
            u.datapath_config[b].alu_out_a_enable = 1
        return u

    seed = base()
    seed.repeat_count = 1
    seed.trigger = (Trigger.COUNT, Trigger.NONE, Trigger.NONE)
    seed.next_uop = (1, 0, 0)
    seed.datapath_config[2].pass_through_alu()
    seed.datapath_config[3].enable_alu(
        AluOp.BYPASS, AluInp.PREV_DELAY_3, AluInp.PREV_DELAY_3
    )
    seed.datapath_config[3].alu_out_a_enable = 1

    steady = base()
    steady.trigger = (Trigger.SRC_TENSOR_DONE, Trigger.NONE, Trigger.NONE)
    steady.require_inp0 = 1
    steady.require_inp1 = 1
    steady.datapath_config[2].enable_alu(
        AluOp.ADD, AluInp.PREV_ALU_OUT, AluInp.PREV_DELAY_0
    )
    steady.datapath_config[3].enable_alu(
        AluOp.ADD, AluInp.CURR_ALU_OUT, AluInp.PREV_ALU_OUT
    )
    steady.datapath_config[3].alu_out_a_enable = 1
    steady.datapath_config[4].swap_enable = 1
    steady.enable_output(OutSel.ALU_OUT, OutPath.WR0_LO)
    steady.enable_output(OutSel.DELAY_1, OutPath.WR0_HI)
    for u in (seed, steady):
        u.validate("v3")
    return [seed, steady]


def _mul_acc_ref(in0, in1, s0, s1, imm2):
    b = (in0.astype(np.float32) * in1.astype(np.float32)).astype(np.float32)
    return b, b.reshape(b.shape[0], -1).sum(axis=-1, keepdims=True)


class _HandUopOp(dve_ops.DveOp):
    """DveOp whose uop programs are hand-written (no sha pin)."""

    def compile(self, ver):
        key = (self.name, ver)
        if (r := dve_ops._COMPILE_CACHE.get(key)) is not None:
            return r
        assert ver == "v3", f"{self.name} authored for TRN2 (v3) only"
        uops = dve_lower(self.spec, ver=ver)
        uops[1].out[OutPath.WR0_LO] = OutSel.ALU_OUT
        result = DveOpSpec(
            name=self.name,
            opcode=dve_ops.get_dve_sub_opcode(self.name),
            uops=uops,
            rd1_en=True,
            uops_2x=_mul_acc_2x_uops(),
        )
        dve_ops._COMPILE_CACHE[key] = result
        return result


def _register(name, spec):
    if name in dve_ops._SUB_OPCODE_FOR_NAME:
        return next(o for o in dve_ops.OPS if o.name == name)
    op = _HandUopOp(name=name, spec=spec, subdim=False, uops_sha={})
    dve_ops.OPS.append(op)
    row = dve_ops._CUSTOM_DVE_ROW_BASE + len(dve_ops.OPS) - 1
    assert row < 0x20
    dve_ops._SUB_OPCODE_FOR_NAME[op.name] = row
    dve_ops.CUSTOM_DVE_SPECS[op.name] = op.spec
    return op


MUL_ACC = _register(
    "MUL_ACC_2X",
    Spec(body=Src0 * Src1, accum=_op_add, accum_init=Zero, reference=_mul_acc_ref),
)


MUL_ACC_PERF_MAX = 1  # 0 = force REGULAR program (debug)


def _emit_custom(nc, op, perf_max, ins_aps, out_ap, rd1_en):
    v = nc.vector
    if op.name not in nc.m.ant_custom_dve_ops:
        nc.m.ant_custom_dve_ops = sorted({*nc.m.ant_custom_dve_ops, op.name})
    shape = bass_isa.CustomDveShape.TTSS
    isa_opcode = nc.isa.Opcode[
        f"NEURON_ISA_TPB_OPCODE_CUSTOM_DVE_ANT_{shape.slot()}"
    ].value
    zero = mybir.ImmediateValue(dtype=mybir.dt.float32, value=0.0)
    return v.add_instruction(
        bass_isa.InstCustomDveAnt(
            name=nc.get_next_instruction_name(),
            op_name=op.name,
            rd1_en=rd1_en,
            subdim=0,
            imm2=0.0,
            shape=shape,
            row=dve_ops.get_dve_sub_opcode(op.name),
            isa_opcode=isa_opcode,
            perf_max=perf_max,
            ins=[v.lower_ap(a, for_isa=True, opt=True) for a in ins_aps]
            + [zero, zero],
            outs=[v.lower_ap(out_ap, for_isa=True, opt=True)],
        )
    )


def emit_mul_acc(nc, out_ap, in0_ap, in1_ap):
    """out_ap[p, 2i] (2x) / out_ap[p, i] (1x fallback) = running total of
    sum_k in0[p, k] * in1[p, k], at 2 elem/cycle/lane in the 2x program."""
    return _emit_custom(
        nc, MUL_ACC, MUL_ACC_PERF_MAX, [in0_ap, in1_ap], out_ap, True
    )


# ---------------------------------------------------------------------------
# Build-time schedule model: pick the warmup matmul count so the PE starts
# the real stream only when the DMA has enough lead to keep it gap-free.

PE_HZ = 2.37e9  # observed full-clock cadence
DMA_BPS = 370e9  # effective HBM read BW per core (measured ~390-410 packed)
MM_OVH = 55  # per-matmul pipelined overhead cycles


def _slot_pe_ns(n_l, msub):
    """PE cycles to consume one slot (stage A + score + mask matmuls)."""
    cyc = 0
    for c0 in range(0, n_l, GCOL):
        w = min(GCOL, n_l) - c0 if c0 + GCOL > n_l else GCOL
        w = min(GCOL, n_l - c0)
        for s0 in range(0, w, SUB):
            sw = min(SUB, w - s0)
            cyc += DC * (sw + MM_OVH)  # stage A
            cyc += sw + MM_OVH  # score
            if (c0 + s0 + sw) > msub * SUB:
                cyc += sw + MM_OVH  # mask
    return cyc / PE_HZ * 1e9


def _warmup_ns(template):
    d_cum = 1300.0  # first HWDGE issue + DGE->DMA delay
    p_cum = 0.0
    t_start = 0.0
    for j, (C, msub) in enumerate(template):
        n_l = C * CHUNK
        d_cum += 128 * DC * (n_l + PADC) * 2 / DMA_BPS * 1e9
        need = d_cum + 900.0 - p_cum  # sem prop after DMA completes
        t_start = max(t_start, need)
        p_cum += _slot_pe_ns(n_l, msub)
    return t_start + 1000.0  # margin: PE arriving late is safe, early is not


def _warmup_count(warm_ns):
    """Number of 512-col warmup matmuls covering warm_ns, given the p-state
    ramp (first ~3us at 1.2GHz, then 2.4GHz)."""
    t, n = 0.0, 0
    while t < warm_ns:
        t += 490.0 if t < 3000.0 else 270.0
        n += 1
    return max(n, 4)


# ---------------------------------------------------------------------------


def _build_bass(template):
    """template: per-slot (n_chunks128, first_mask_sub512) pairs, len BL."""
    nc = bacc.Bacc(
        "TRN2", target_bir_lowering=False, debug=False, num_devices=N_CORES
    )
    sec_w = [DC * (C * CHUNK + PADC) for C, _ in template]
    sec_off = np.concatenate([[0], np.cumsum(sec_w)]).astype(int)
    totc = int(sec_off[-1])

    encP = nc.dram_tensor("encP", [128, totc], F16, kind="ExternalInput")
    msk = nc.dram_tensor("msk", [1, BL * L], F8, kind="ExternalInput")
    hplus = nc.dram_tensor("hplus", [A, BL], F32, kind="ExternalInput")
    w_enc = nc.dram_tensor("w_enc", [D, A], F16, kind="ExternalInput")
    vrep = nc.dram_tensor("vrep", [A, 128], F16, kind="ExternalInput")
    ones1 = nc.dram_tensor("ones1", [1, 128], F8, kind="ExternalInput")
    ident = nc.dram_tensor("ident", [128, 128], F32, kind="ExternalInput")
    out = nc.dram_tensor("out", [BL * DC, 128], F32, kind="ExternalOutput")
    outS = nc.dram_tensor("outS", [1, BL], F32, kind="ExternalOutput")

    with tile.TileContext(nc) as tc, ExitStack() as ctx:
        const = ctx.enter_context(tc.tile_pool(name="const", bufs=1))
        encp = ctx.enter_context(tc.tile_pool(name="encp", bufs=7))
        tanhp = ctx.enter_context(tc.tile_pool(name="tanhp", bufs=3))
        pp = ctx.enter_context(tc.tile_pool(name="pp", bufs=4))
        scrp = ctx.enter_context(tc.tile_pool(name="scrp", bufs=2))
        smallp = ctx.enter_context(tc.tile_pool(name="smallp", bufs=4))
        resp = ctx.enter_context(tc.tile_pool(name="resp", bufs=1))
        ps_parts = ctx.enter_context(
            tc.tile_pool(name="ps_parts", bufs=2, space="PSUM")
        )
        ps_sc = ctx.enter_context(tc.tile_pool(name="ps_sc", bufs=2, space="PSUM"))

        # ---- w_enc rides FIRST on the sync queue (64KB, ~0.3us) so stage A
        # is never gated on the contended const queues; then the enc stream,
        # one contiguous transfer per slot ----
        w_enc_sb = const.tile([128, DC, A], F16, tag="c_w_enc")
        nc.sync.dma_start(
            w_enc_sb[:], w_enc.ap().rearrange("(dc p) a -> p dc a", p=128)
        )
        ets = []
        for j, (C, msub) in enumerate(template):
            n_l = C * CHUNK
            et = encp.tile([128, DC, n_l + PADC], F16, tag="et")
            nc.sync.dma_start(
                et[:],
                encP.ap()[:, int(sec_off[j]) : int(sec_off[j + 1])].rearrange(
                    "p (dc l) -> p dc l", dc=DC
                ),
            )
            ets.append(et)

        # ---- remaining constants split across the two idle HWDGE queues,
        # critical-first (hplus gates the first tanh, msk the first score) --
        def loaded(shape, dtype, dram_ap, eng):
            dst = const.tile(shape, dtype, tag="c_" + dram_ap.tensor.name)
            eng.dma_start(dst[:], dram_ap)
            return dst

        hplus_sb = loaded([A, BL], F32, hplus.ap(), nc.gpsimd)
        vrep_sb = loaded([A, 128], F16, vrep.ap(), nc.gpsimd)
        msk_sb = loaded([1, BL * L], F8, msk.ap(), nc.scalar)
        ones1_sb = loaded([1, 128], F8, ones1.ap(), nc.scalar)
        ident_sb = loaded([128, 128], F32, ident.ap(), nc.scalar)

        # ---- PE warmup: ramp the PE clock (short ~1us dependency stalls in
        # the later stream do NOT drop the p-state, so the warmup only needs
        # to cover the ramp window, not the full DMA lead) ----
        n_warm = 14
        warm_sb = const.tile([128, SUB], F16, tag="warm_in")
        nc.vector.memset(warm_sb[:], 0.0)
        for wi in range(n_warm):
            warm_ps = ps_sc.tile([128, GCOL], F32, tag="sc")
            nc.tensor.matmul(
                warm_ps[:, 0:SUB], lhsT=warm_sb[:, 0:128], rhs=warm_sb[:],
                start=True, stop=True,
            )

        res = resp.tile([128, BL * DC], F32)  # col j*DC+dc <- accum_out
        s_all = resp.tile([128, BL], F32)  # softmax denominators

        # ---- flat group list for the software-pipelined emission ----
        groups = []  # (slot j, c0, c1, local_gi, is_last)
        for j, (C, msub) in enumerate(template):
            n_l = C * CHUNK
            gs = [(g, min(g + GCOL, n_l)) for g in range(0, n_l, GCOL)]
            for gi, (c0, c1) in enumerate(gs):
                groups.append((j, c0, c1, gi, gi == len(gs) - 1))

        p_tiles = {}
        sh_tiles = {}
        parts_tiles = {}
        sc_tiles = {}
        th_tiles = {}

        def emit_stage_a(g):
            j, c0, c1, gi, _ = groups[g]
            C, msub = template[j]
            n_l = C * CHUNK
            et = ets[j]
            if gi == 0:
                p_sb = pp.tile([128, n_l + PADC], BF16, tag="p")
                nc.gpsimd.memset(p_sb[:, n_l : n_l + PADC], 0.0)
                p_tiles[j] = p_sb
            w = c1 - c0
            parts_ps = ps_parts.tile([A, GCOL], F32, tag="parts")
            parts_tiles[g] = parts_ps
            for dc in range(DC):
                for s0 in range(0, w, SUB):
                    sw = min(SUB, w - s0)
                    nc.tensor.matmul(
                        parts_ps[:, s0 : s0 + sw],
                        lhsT=w_enc_sb[:, dc, :],
                        rhs=et[:, dc, c0 + s0 : c0 + s0 + sw],
                        start=(dc == 0), stop=(dc == DC - 1),
                    )

        def emit_tanh(g):
            j, c0, c1, gi, _ = groups[g]
            w = c1 - c0
            th = tanhp.tile([A, GCOL], F16, tag="th")
            th_tiles[g] = th
            nc.scalar.activation(
                th[:, 0:w], parts_tiles.pop(g)[:, 0:w],
                mybir.ActivationFunctionType.Tanh,
                bias=hplus_sb[:, j : j + 1],
            )

        def emit_score(g):
            j, c0, c1, gi, _ = groups[g]
            C, msub = template[j]
            w = c1 - c0
            th = th_tiles.pop(g)
            sc_ps = ps_sc.tile([128, GCOL], F32, tag="sc")
            sc_tiles[g] = sc_ps
            for s0 in range(0, w, SUB):
                sw = min(SUB, w - s0)
                has_mask = (c0 + s0 + sw) > msub * SUB
                # mask first: it has no tanh dependency, so it fills the PE
                # while the ACT finishes tanh for this group
                if has_mask:
                    nc.tensor.matmul(
                        sc_ps[:, s0 : s0 + sw], lhsT=ones1_sb[:],
                        rhs=msk_sb[:, j * L + c0 + s0 : j * L + c0 + s0 + sw],
                        start=True, stop=False,
                    )
                nc.tensor.matmul(
                    sc_ps[:, s0 : s0 + sw], lhsT=vrep_sb[:],
                    rhs=th[:, s0 : s0 + sw],
                    start=not has_mask, stop=True,
                )

        def emit_exp(g):
            j, c0, c1, gi, last = groups[g]
            C, msub = template[j]
            n_l = C * CHUNK
            w = c1 - c0
            sh = smallp.tile([128, 1], F32, tag=f"sh{gi}")
            nc.scalar.activation(
                p_tiles[j][:, c0:c1], sc_tiles.pop(g)[:, 0:w],
                mybir.ActivationFunctionType.Exp, accum_out=sh[:],
            )
            sh_tiles.setdefault(j, []).append(sh)
            if last:
                emit_slot_tail(j, n_l)

        def emit_slot_tail(j, n_l):
            s_parts = sh_tiles.pop(j)
            if len(s_parts) == 2:
                nc.gpsimd.tensor_add(
                    s_all[:, j : j + 1], s_parts[0][:], s_parts[1][:]
                )
            else:
                nc.gpsimd.tensor_copy(s_all[:, j : j + 1], s_parts[0][:])
            et = ets[j]
            p_sb = p_tiles.pop(j)
            scr = scrp.tile([128, DC, n_l + PADC], BF16, tag="scr")
            for dc in range(DC):
                emit_mul_acc(
                    nc,
                    scr[:, dc, 0 : n_l + PADC],
                    et[:, dc, 0 : n_l + PADC],
                    p_sb[:, 0 : n_l + PADC],
                )
            nc.gpsimd.tensor_copy(
                res[:, j * DC : (j + 1) * DC], scr[:, :, n_l + PADC - 4]
            )

        # software pipeline with a 2-group score lag: the score matmul for
        # group g issues two stage-A groups later, so it never waits on the
        # ACT's tanh; exp trails one more group.
        G = len(groups)
        for g in range(G + 3):
            if g < G:
                emit_stage_a(g)
            if 1 <= g <= G:
                emit_tanh(g - 1)
            if 2 <= g <= G + 1:
                emit_score(g - 2)
            if g >= 3:
                emit_exp(g - 3)

        # transpose + write out in two parts (host does the normalization):
        # part A (slots 0..5) is ready before the last slots finish, so its
        # DMA overlaps the drain; PSUM is DMA'd directly (no SBUF bounce).
        cutA = 6 * DC
        nB = BL * DC - cutA
        out_sbA = resp.tile([cutA, 128], F32)
        t_a = ps_parts.tile([A, GCOL], F32, tag="parts")
        nc.tensor.transpose(t_a[0:cutA, 0:128], res[:, 0:cutA], ident_sb[:])
        nc.scalar.copy(out_sbA[:], t_a[0:cutA, 0:128])
        nc.sync.dma_start(out.ap()[0:cutA, :], out_sbA[:])
        out_sbB = resp.tile([nB, 128], F32)
        t_b = ps_parts.tile([A, GCOL], F32, tag="parts")
        nc.tensor.transpose(t_b[0:nB, 0:128], res[:, cutA : BL * DC], ident_sb[:])
        nc.scalar.copy(out_sbB[:], t_b[0:nB, 0:128])
        nc.sync.dma_start(out.ap()[cutA : BL * DC, :], out_sbB[:])
        nc.sync.dma_start(outS.ap(), s_all[0:1, :])

    nc.compile()
    return nc


_NC_CACHE = {}


def _get_nc(template):
    key = tuple((int(c), int(m)) for c, m in template)
    if key not in _NC_CACHE:
        _NC_CACHE[key] = _build_bass(key)
    return _NC_CACHE[key]


def _plan(lens):
    """Balance batches across cores by valid-chunk count.

    Returns (assign, template): assign[c][j] = original batch index handled
    by core c, slot j; template[j] = (chunks, first_mask_sub) compiled for
    slot j. Slots descend in size so the drain tail is short.
    """
    lens = np.maximum(np.asarray(lens), 1)
    chunks = np.minimum(np.ceil(lens / CHUNK).astype(int), NCH)
    order = np.argsort(-chunks, kind="stable")  # descending need
    # slot order: a medium group first (its DMA lands right as the PE
    # warmup ends, and its exp starts the DVE stream early), then the big
    # groups (the DVE builds a backlog and runs gap-free), smallest last
    # (short drain tail).
    ranks = list(range(BL))
    mid = BL // 2
    slot_ranks = [ranks[mid]] + ranks[:mid] + ranks[mid + 1 :]
    assign = [
        [int(order[r * N_CORES + c]) for r in slot_ranks] for c in range(N_CORES)
    ]
    template = []
    for r in slot_ranks:
        group = [int(order[r * N_CORES + c]) for c in range(N_CORES)]
        cmax = int(chunks[order[r * N_CORES]])
        min_len = int(min(lens[b] for b in group))
        template.append((cmax, min_len // SUB))
    return assign, tuple(template)


def prepare_in_maps(enc_outputs, lens, hidden_states, W_enc, b_attn, W_hidden, v):
    """Host-side sharding + layout transforms. Returns (in_maps, assign, t)."""
    enc_outputs = np.asarray(enc_outputs, dtype=np.float32)
    lens = np.asarray(lens, dtype=np.int32)
    hidden_states = np.asarray(hidden_states, dtype=np.float32)
    W_enc = np.asarray(W_enc, dtype=np.float32)
    b_attn = np.asarray(b_attn, dtype=np.float32)
    W_hidden = np.asarray(W_hidden, dtype=np.float32)
    v = np.asarray(v, dtype=np.float32)

    assign, template = _plan(lens)

    # (L, B, D) -> (B, D, L) contiguous fp16 (halves the HBM traffic)
    encT = np.ascontiguousarray(enc_outputs.transpose(1, 2, 0).astype(np.float16))
    w_enc_r = W_enc.astype(np.float16)
    vrep = np.ascontiguousarray(np.repeat(v.astype(np.float16)[:, None], 128, axis=1))
    ones1 = np.ones((1, 128), dtype=ml_dtypes.float8_e5m2)
    ident = np.eye(128, dtype=np.float32)
    b_attn_c = np.ascontiguousarray(b_attn[:, None])

    # length mask rows: 0 where l < lens[b], -30000 where l >= lens[b]
    li = np.arange(L, dtype=np.int32)[None, :]
    mask_full = np.where(li < lens[:, None], 0.0, -30000.0).astype(
        ml_dtypes.float8_e5m2
    )  # (B, L)

    # per-partition tanh bias, computed on host: b_attn + (hidden@W_hidden).T
    hplus_all = (hidden_states @ W_hidden).T + b_attn_c  # (A, B)

    in_maps = []
    for c in range(N_CORES):
        bs = assign[c]
        # packed enc: per slot, [128, DC*(n_l+PADC)] with zero pads
        secs = []
        for j, (C, _) in enumerate(template):
            n_l = C * CHUNK
            eb = encT[bs[j]].reshape(DC, 128, L)[:, :, :n_l]  # (DC,128,n_l)
            ebp = np.zeros((DC, 128, n_l + PADC), dtype=np.float16)
            ebp[:, :, :n_l] = eb
            secs.append(ebp.transpose(1, 0, 2).reshape(128, DC * (n_l + PADC)))
        encP = np.ascontiguousarray(np.concatenate(secs, axis=1))
        in_maps.append(
            {
                "encP": encP,
                "msk": np.ascontiguousarray(mask_full[bs]).reshape(1, BL * L),
                "hplus": np.ascontiguousarray(hplus_all[:, bs]),
                "w_enc": w_enc_r,
                "vrep": vrep,
                "ones1": ones1,
                "ident": ident,
            }
        )
    return in_maps, assign, template


def _run(inputs_np, trace=False):
    in_maps, assign, template = prepare_in_maps(**inputs_np)
    nc = _get_nc(template)
    res = run_bass_kernel_spmd(
        nc, in_maps, core_ids=list(range(N_CORES)), trace=trace
    )
    out = np.empty((B, D), dtype=np.float32)
    for c in range(N_CORES):
        rows = res.results[c]["out"].reshape(BL, D)
        s = res.results[c]["outS"].reshape(BL)
        for j in range(BL):
            out[assign[c][j]] = rows[j] / s[j]
    return out, res


def kernel(enc_outputs, lens, hidden_states, W_enc, b_attn, W_hidden, v, **kwargs):
    out, _ = _run(
        dict(
            enc_outputs=enc_outputs, lens=lens, hidden_states=hidden_states,
            W_enc=W_enc, b_attn=b_attn, W_hidden=W_hidden, v=v,
        )
    )
    return out


def kernel_traced(enc_outputs, lens, hidden_states, W_enc, b_attn, W_hidden, v):
    """Like kernel() but returns (output, BassKernelResults with trace)."""
    return _run(
        dict(
            enc_outputs=enc_outputs, lens=lens, hidden_states=hidden_states,
            W_enc=W_enc, b_attn=b_attn, W_hidden=W_hidden, v=v,
        ),
        trace=True,
    )


# revision 42
# speedup vs baseline: 1.0809x; 1.0809x over previous
"""Bahdanau additive attention (ragged sequence) on 8 Trainium2 NeuronCores.

Reference math (per batch b over sequence l, d=512, a=64):
    parts  = enc @ W_enc + b_attn                        (l, a)
    scores = tanh(parts + hidden @ W_hidden) . v         (l,)
    w      = softmax(scores + mask) over l               (valid: l < lens[b])
    out[b] = sum_l w[l] * enc[l, b, :]                   (512,)

Strategy (batch-parallel over 8 cores, 8 batches each; single pass over enc):
  * Host packs enc per (slot, d-chunk) into one contiguous fp16 buffer
    [128, sum_j 4*(n_l_j+16)] including 16 zero-pad columns per section, so
    each slot is ONE sync-queue DMA with ~16KB/partition contiguous rows
    (max descriptor efficiency, 1 HWDGE instruction per slot instead of 4).
  * Ragged skipping at 128-column granularity (positions l >= lens[b]
    contribute exactly 0); host sorts batches by chunk count, deals them
    across cores, compiles against the per-slot max-chunk template.
  * The PE p-state drops to 1.2 GHz on any idle gap and needs ~3us of
    continuous work to regain 2.4 GHz. So: a computed run of warmup matmuls
    keeps the PE hot until the DMA stream has enough lead that the real
    matmul stream (stage A + score) then runs with zero dependency gaps.
    Emission is software-pipelined: stage A of group g+1 is issued between
    stage A(g) and score(g) so the tanh (ACT) latency is hidden.
  * Scores group 1024 columns per PSUM tile: one tanh + one exp per group.
    Length mask (host-built 0/-30000 fp8 rows) accumulates into the score
    PSUM via an extra PE matmul only for 512-col subchunks past the slot's
    min valid length. exp runs on ACT out of PSUM to BF16 with accum_out
    giving the softmax denominator for free (exp(-30000) == 0 exactly).
  * Stage B: custom DVE op (2 elem/cycle/lane packed 16-bit) computes the
    running total of et*p per (slot, d-chunk); the settled total is read
    from the zero-padded tail. Softmax normalization happens on HOST (the
    kernel outputs unnormalized sums R and denominators S).
"""
import sys

sys.path.insert(0, "/opt/trn_rl_repo")

from contextlib import ExitStack

import ml_dtypes
import numpy as np

import concourse.bacc as bacc
import concourse.bass as bass  # noqa: F401  (kept for debugging)
import concourse.bass_isa as bass_isa
import concourse.dve_ops as dve_ops
import concourse.tile as tile
from concourse import mybir
from concourse.bass_utils import run_bass_kernel_spmd
from concourse.dve_spec import Spec, Src0, Src1, Zero, lower as dve_lower
from concourse.dve_uop import (
    AluInp,
    AluOp,
    DelayInp,
    DveOpSpec,
    InpSel,
    OutPath,
    OutSel,
    Trigger,
    UopConfig,
)
from operator import add as _op_add

F32 = mybir.dt.float32
F16 = mybir.dt.float16
BF16 = mybir.dt.bfloat16
F8 = mybir.dt.float8e5

N_CORES = 8
L, B, D, A, H = 2048, 64, 512, 64, 512
BL = B // N_CORES  # local batches per core
DC = D // 128  # d-chunks of 128 partitions
CHUNK = 128  # l-chunk width for ragged skipping
GCOL = 1024  # columns per PSUM score/parts group (2 banks)
SUB = 512  # matmul moving-operand width
NCH = L // CHUNK
PADC = 16  # zero-padded tail columns for the MUL_ACC_2X ripple readback

# ---------------------------------------------------------------------------
# MUL_ACC_2X: custom DVE op computing accum_out[p] = sum_k in0[p,k]*in1[p,k]
# in one pass at 2 elem/cycle/lane for 16-bit operands. See baseline notes:
# running totals ripple down the ALU chain; the settled total is read from
# the zero-padded tail region (col n-4 of a 16-padded stream).


def _mul_acc_2x_uops():
    lanes = [InpSel.SRC_0, InpSel.SRC_1, InpSel.SRC_0_HI, InpSel.SRC_1_HI,
             InpSel.ZERO]

    def base():
        u = UopConfig()
        for i, s in enumerate(lanes):
            u.enable_input(s, i)
        u.accum_enabled = 1
        u.datapath_config[0].enable_alu(
            AluOp.MULTIPLY, AluInp.PREV_ALU_OUT, AluInp.PREV_DELAY_0
        ).pass_through_delay(1, 2, 3)
        u.datapath_config[1].enable_alu(
            AluOp.MULTIPLY, AluInp.PREV_DELAY_1, AluInp.PREV_DELAY_2
        ).enable_delay_from_src(DelayInp.PREV_ALU_OUT, 0).pass_through_delay(3)
        u.datapath_config[2].enable_delay_from_src(
            DelayInp.PREV_ALU_OUT, 1
        ).pass_through_delay(0, 3)
        for b in range(3, 8):
            u.datapath_config[b].pass_through_delay(0, 1)
        for b in range(4, 8):
            u.datapath_config[b].pass_through_alu()
            u.datapath_config[b].alu_out_a_enable = 1
        return u

    seed = base()
    seed.repeat_count = 1
    seed.trigger = (Trigger.COUNT, Trigger.NONE, Trigger.NONE)
    seed.next_uop = (1, 0, 0)
    seed.datapath_config[2].pass_through_alu()
    seed.datapath_config[3].enable_alu(
        AluOp.BYPASS, AluInp.PREV_DELAY_3, AluInp.PREV_DELAY_3
    )
    seed.datapath_config[3].alu_out_a_enable = 1

    steady = base()
    steady.trigger = (Trigger.SRC_TENSOR_DONE, Trigger.NONE, Trigger.NONE)
    steady.require_inp0 = 1
    steady.require_inp1 = 1
    steady.datapath_config[2].enable_alu(
        AluOp.ADD, AluInp.PREV_ALU_OUT, AluInp.PREV_DELAY_0
    )
    steady.datapath_config[3].enable_alu(
        AluOp.ADD, AluInp.CURR_ALU_OUT, AluInp.PREV_ALU_OUT
    )
    steady.datapath_config[3].alu_out_a_enable = 1
    steady.datapath_config[4].swap_enable = 1
    steady.enable_output(OutSel.ALU_OUT, OutPath.WR0_LO)
    steady.enable_output(OutSel.DELAY_1, OutPath.WR0_HI)
    for u in (seed, steady):
        u.validate("v3")
    return [seed, steady]


def _mul_acc_ref(in0, in1, s0, s1, imm2):
    b = (in0.astype(np.float32) * in1.astype(np.float32)).astype(np.float32)
    return b, b.reshape(b.shape[0], -1).sum(axis=-1, keepdims=True)


class _HandUopOp(dve_ops.DveOp):
    """DveOp whose uop programs are hand-written (no sha pin)."""

    def compile(self, ver):
        key = (self.name, ver)
        if (r := dve_ops._COMPILE_CACHE.get(key)) is not None:
            return r
        assert ver == "v3", f"{self.name} authored for TRN2 (v3) only"
        uops = dve_lower(self.spec, ver=ver)
        uops[1].out[OutPath.WR0_LO] = OutSel.ALU_OUT
        result = DveOpSpec(
            name=self.name,
            opcode=dve_ops.get_dve_sub_opcode(self.name),
            uops=uops,
            rd1_en=True,
            uops_2x=_mul_acc_2x_uops(),
        )
        dve_ops._COMPILE_CACHE[key] = result
        return result


def _register(name, spec):
    if name in dve_ops._SUB_OPCODE_FOR_NAME:
        return next(o for o in dve_ops.OPS if o.name == name)
    op = _HandUopOp(name=name, spec=spec, subdim=False, uops_sha={})
    dve_ops.OPS.append(op)
    row = dve_ops._CUSTOM_DVE_ROW_BASE + len(dve_ops.OPS) - 1
    assert row < 0x20
    dve_ops._SUB_OPCODE_FOR_NAME[op.name] = row
    dve_ops.CUSTOM_DVE_SPECS[op.name] = op.spec
    return op


MUL_ACC = _register(
    "MUL_ACC_2X",
    Spec(body=Src0 * Src1, accum=_op_add, accum_init=Zero, reference=_mul_acc_ref),
)


MUL_ACC_PERF_MAX = 1  # 0 = force REGULAR program (debug)


def _emit_custom(nc, op, perf_max, ins_aps, out_ap, rd1_en):
    v = nc.vector
    if op.name not in nc.m.ant_custom_dve_ops:
        nc.m.ant_custom_dve_ops = sorted({*nc.m.ant_custom_dve_ops, op.name})
    shape = bass_isa.CustomDveShape.TTSS
    isa_opcode = nc.isa.Opcode[
        f"NEURON_ISA_TPB_OPCODE_CUSTOM_DVE_ANT_{shape.slot()}"
    ].value
    zero = mybir.ImmediateValue(dtype=mybir.dt.float32, value=0.0)
    return v.add_instruction(
        bass_isa.InstCustomDveAnt(
            name=nc.get_next_instruction_name(),
            op_name=op.name,
            rd1_en=rd1_en,
            subdim=0,
            imm2=0.0,
            shape=shape,
            row=dve_ops.get_dve_sub_opcode(op.name),
            isa_opcode=isa_opcode,
            perf_max=perf_max,
            ins=[v.lower_ap(a, for_isa=True, opt=True) for a in ins_aps]
            + [zero, zero],
            outs=[v.lower_ap(out_ap, for_isa=True, opt=True)],
        )
    )


def emit_mul_acc(nc, out_ap, in0_ap, in1_ap):
    """out_ap[p, 2i] (2x) / out_ap[p, i] (1x fallback) = running total of
    sum_k in0[p, k] * in1[p, k], at 2 elem/cycle/lane in the 2x program."""
    return _emit_custom(
        nc, MUL_ACC, MUL_ACC_PERF_MAX, [in0_ap, in1_ap], out_ap, True
    )


# ---------------------------------------------------------------------------
# Build-time schedule model: pick the warmup matmul count so the PE starts
# the real stream only when the DMA has enough lead to keep it gap-free.

PE_HZ = 2.37e9  # observed full-clock cadence
DMA_BPS = 370e9  # effective HBM read BW per core (measured ~390-410 packed)
MM_OVH = 55  # per-matmul pipelined overhead cycles


def _slot_pe_ns(n_l, msub):
    """PE cycles to consume one slot (stage A + score + mask matmuls)."""
    cyc = 0
    for c0 in range(0, n_l, GCOL):
        w = min(GCOL, n_l) - c0 if c0 + GCOL > n_l else GCOL
        w = min(GCOL, n_l - c0)
        for s0 in range(0, w, SUB):
            sw = min(SUB, w - s0)
            cyc += DC * (sw + MM_OVH)  # stage A
            cyc += sw + MM_OVH  # score
            if (c0 + s0 + sw) > msub * SUB:
                cyc += sw + MM_OVH  # mask
    return cyc / PE_HZ * 1e9


def _warmup_ns(template):
    d_cum = 1300.0  # first HWDGE issue + DGE->DMA delay
    p_cum = 0.0
    t_start = 0.0
    for j, (C, msub) in enumerate(template):
        n_l = C * CHUNK
        d_cum += 128 * DC * (n_l + PADC) * 2 / DMA_BPS * 1e9
        need = d_cum + 900.0 - p_cum  # sem prop after DMA completes
        t_start = max(t_start, need)
        p_cum += _slot_pe_ns(n_l, msub)
    return t_start + 1000.0  # margin: PE arriving late is safe, early is not


def _warmup_count(warm_ns):
    """Number of 512-col warmup matmuls covering warm_ns, given the p-state
    ramp (first ~3us at 1.2GHz, then 2.4GHz)."""
    t, n = 0.0, 0
    while t < warm_ns:
        t += 490.0 if t < 3000.0 else 270.0
        n += 1
    return max(n, 4)


# ---------------------------------------------------------------------------


def _build_bass(template):
    """template: per-slot (n_chunks128, first_mask_sub512) pairs, len BL."""
    nc = bacc.Bacc(
        "TRN2", target_bir_lowering=False, debug=False, num_devices=N_CORES
    )
    sec_w = [DC * (C * CHUNK + PADC) for C, _ in template]
    sec_off = np.concatenate([[0], np.cumsum(sec_w)]).astype(int)
    totc = int(sec_off[-1])

    encP = nc.dram_tensor("encP", [128, totc], F16, kind="ExternalInput")
    msk = nc.dram_tensor("msk", [1, BL * L], F8, kind="ExternalInput")
    hplus = nc.dram_tensor("hplus", [A, BL], F32, kind="ExternalInput")
    w_enc = nc.dram_tensor("w_enc", [D, A], F16, kind="ExternalInput")
    vrep = nc.dram_tensor("vrep", [A, 128], F16, kind="ExternalInput")
    ones1 = nc.dram_tensor("ones1", [1, 128], F8, kind="ExternalInput")
    ident = nc.dram_tensor("ident", [128, 128], F32, kind="ExternalInput")
    out = nc.dram_tensor("out", [BL * DC, 128], F32, kind="ExternalOutput")
    outS = nc.dram_tensor("outS", [1, BL], F32, kind="ExternalOutput")

    with tile.TileContext(nc) as tc, ExitStack() as ctx:
        const = ctx.enter_context(tc.tile_pool(name="const", bufs=1))
        encp = ctx.enter_context(tc.tile_pool(name="encp", bufs=7))
        tanhp = ctx.enter_context(tc.tile_pool(name="tanhp", bufs=3))
        pp = ctx.enter_context(tc.tile_pool(name="pp", bufs=4))
        scrp = ctx.enter_context(tc.tile_pool(name="scrp", bufs=2))
        smallp = ctx.enter_context(tc.tile_pool(name="smallp", bufs=4))
        resp = ctx.enter_context(tc.tile_pool(name="resp", bufs=1))
        ps_parts = ctx.enter_context(
            tc.tile_pool(name="ps_parts", bufs=2, space="PSUM")
        )
        ps_sc = ctx.enter_context(tc.tile_pool(name="ps_sc", bufs=2, space="PSUM"))

        # ---- w_enc rides FIRST on the sync queue (64KB, ~0.3us) so stage A
        # is never gated on the contended const queues; then the enc stream,
        # one contiguous transfer per slot ----
        w_enc_sb = const.tile([128, DC, A], F16, tag="c_w_enc")
        nc.sync.dma_start(
            w_enc_sb[:], w_enc.ap().rearrange("(dc p) a -> p dc a", p=128)
        )
        ets = []
        for j, (C, msub) in enumerate(template):
            n_l = C * CHUNK
            et = encp.tile([128, DC, n_l + PADC], F16, tag="et")
            nc.sync.dma_start(
                et[:],
                encP.ap()[:, int(sec_off[j]) : int(sec_off[j + 1])].rearrange(
                    "p (dc l) -> p dc l", dc=DC
                ),
            )
            ets.append(et)

        # ---- remaining constants split across the two idle HWDGE queues,
        # critical-first (hplus gates the first tanh, msk the first score) --
        def loaded(shape, dtype, dram_ap, eng):
            dst = const.tile(shape, dtype, tag="c_" + dram_ap.tensor.name)
            eng.dma_start(dst[:], dram_ap)
            return dst

        hplus_sb = loaded([A, BL], F32, hplus.ap(), nc.gpsimd)
        vrep_sb = loaded([A, 128], F16, vrep.ap(), nc.gpsimd)
        msk_sb = loaded([1, BL * L], F8, msk.ap(), nc.scalar)
        ones1_sb = loaded([1, 128], F8, ones1.ap(), nc.scalar)
        ident_sb = loaded([128, 128], F32, ident.ap(), nc.scalar)

        # ---- PE warmup: ramp the PE clock (short ~1us dependency stalls in
        # the later stream do NOT drop the p-state, so the warmup only needs
        # to cover the ramp window, not the full DMA lead) ----
        n_warm = 14
        warm_sb = const.tile([128, SUB], F16, tag="warm_in")
        nc.vector.memset(warm_sb[:], 0.0)
        for wi in range(n_warm):
            warm_ps = ps_sc.tile([128, GCOL], F32, tag="sc")
            nc.tensor.matmul(
                warm_ps[:, 0:SUB], lhsT=warm_sb[:, 0:128], rhs=warm_sb[:],
                start=True, stop=True,
            )

        res = resp.tile([128, BL * DC], F32)  # col j*DC+dc <- accum_out
        s_all = resp.tile([128, BL], F32)  # softmax denominators

        # ---- flat group list for the software-pipelined emission ----
        groups = []  # (slot j, c0, c1, local_gi, is_last)
        for j, (C, msub) in enumerate(template):
            n_l = C * CHUNK
            gs = [(g, min(g + GCOL, n_l)) for g in range(0, n_l, GCOL)]
            for gi, (c0, c1) in enumerate(gs):
                groups.append((j, c0, c1, gi, gi == len(gs) - 1))

        p_tiles = {}
        sh_tiles = {}
        parts_tiles = {}
        sc_tiles = {}
        th_tiles = {}

        def emit_stage_a(g):
            j, c0, c1, gi, _ = groups[g]
            C, msub = template[j]
            n_l = C * CHUNK
            et = ets[j]
            if gi == 0:
                p_sb = pp.tile([128, n_l + PADC], BF16, tag="p")
                nc.gpsimd.memset(p_sb[:, n_l : n_l + PADC], 0.0)
                p_tiles[j] = p_sb
            w = c1 - c0
            parts_ps = ps_parts.tile([A, GCOL], F32, tag="parts")
            parts_tiles[g] = parts_ps
            for dc in range(DC):
                for s0 in range(0, w, SUB):
                    sw = min(SUB, w - s0)
                    nc.tensor.matmul(
                        parts_ps[:, s0 : s0 + sw],
                        lhsT=w_enc_sb[:, dc, :],
                        rhs=et[:, dc, c0 + s0 : c0 + s0 + sw],
                        start=(dc == 0), stop=(dc == DC - 1),
                    )

        def emit_tanh(g):
            j, c0, c1, gi, _ = groups[g]
            w = c1 - c0
            th = tanhp.tile([A, GCOL], F16, tag="th")
            th_tiles[g] = th
            nc.scalar.activation(
                th[:, 0:w], parts_tiles.pop(g)[:, 0:w],
                mybir.ActivationFunctionType.Tanh,
                bias=hplus_sb[:, j : j + 1],
            )

        def emit_score(g):
            j, c0, c1, gi, _ = groups[g]
            C, msub = template[j]
            w = c1 - c0
            th = th_tiles.pop(g)
            sc_ps = ps_sc.tile([128, GCOL], F32, tag="sc")
            sc_tiles[g] = sc_ps
            for s0 in range(0, w, SUB):
                sw = min(SUB, w - s0)
                has_mask = (c0 + s0 + sw) > msub * SUB
                # mask first: it has no tanh dependency, so it fills the PE
                # while the ACT finishes tanh for this group
                if has_mask:
                    nc.tensor.matmul(
                        sc_ps[:, s0 : s0 + sw], lhsT=ones1_sb[:],
                        rhs=msk_sb[:, j * L + c0 + s0 : j * L + c0 + s0 + sw],
                        start=True, stop=False,
                    )
                nc.tensor.matmul(
                    sc_ps[:, s0 : s0 + sw], lhsT=vrep_sb[:],
                    rhs=th[:, s0 : s0 + sw],
                    start=not has_mask, stop=True,
                )

        def emit_exp(g):
            j, c0, c1, gi, last = groups[g]
            C, msub = template[j]
            n_l = C * CHUNK
            w = c1 - c0
            sh = smallp.tile([128, 1], F32, tag=f"sh{gi}")
            nc.scalar.activation(
                p_tiles[j][:, c0:c1], sc_tiles.pop(g)[:, 0:w],
                mybir.ActivationFunctionType.Exp, accum_out=sh[:],
            )
            sh_tiles.setdefault(j, []).append(sh)
            if last:
                emit_slot_tail(j, n_l)

        def emit_slot_tail(j, n_l):
            s_parts = sh_tiles.pop(j)
            if len(s_parts) == 2:
                nc.gpsimd.tensor_add(
                    s_all[:, j : j + 1], s_parts[0][:], s_parts[1][:]
                )
            else:
                nc.gpsimd.tensor_copy(s_all[:, j : j + 1], s_parts[0][:])
            et = ets[j]
            p_sb = p_tiles.pop(j)
            scr = scrp.tile([128, DC, n_l + PADC], BF16, tag="scr")
            for dc in range(DC):
                emit_mul_acc(
                    nc,
                    scr[:, dc, 0 : n_l + PADC],
                    et[:, dc, 0 : n_l + PADC],
                    p_sb[:, 0 : n_l + PADC],
                )
            nc.gpsimd.tensor_copy(
                res[:, j * DC : (j + 1) * DC], scr[:, :, n_l + PADC - 4]
            )

        # software pipeline with a 2-group score lag: the score matmul for
        # group g issues two stage-A groups later, so it never waits on the
        # ACT's tanh; exp trails one more group.
        G = len(groups)
        for g in range(G + 3):
            if g < G:
                emit_stage_a(g)
            if 1 <= g <= G:
                emit_tanh(g - 1)
            if 2 <= g <= G + 1:
                emit_score(g - 2)
            if g >= 3:
                emit_exp(g - 3)

        # transpose + write out in two parts (host does the normalization):
        # part A (slots 0..5) is ready before the last slots finish, so its
        # DMA overlaps the drain; PSUM is DMA'd directly (no SBUF bounce).
        cutA = 6 * DC
        nB = BL * DC - cutA
        out_sbA = resp.tile([cutA, 128], F32)
        t_a = ps_parts.tile([A, GCOL], F32, tag="parts")
        nc.tensor.transpose(t_a[0:cutA, 0:128], res[:, 0:cutA], ident_sb[:])
        nc.scalar.copy(out_sbA[:], t_a[0:cutA, 0:128])
        nc.sync.dma_start(out.ap()[0:cutA, :], out_sbA[:])
        out_sbB = resp.tile([nB, 128], F32)
        t_b = ps_parts.tile([A, GCOL], F32, tag="parts")
        nc.tensor.transpose(t_b[0:nB, 0:128], res[:, cutA : BL * DC], ident_sb[:])
        nc.scalar.copy(out_sbB[:], t_b[0:nB, 0:128])
        nc.sync.dma_start(out.ap()[cutA : BL * DC, :], out_sbB[:])
        nc.sync.dma_start(outS.ap(), s_all[0:1, :])

    nc.compile()
    return nc


_NC_CACHE = {}


def _get_nc(template):
    key = tuple((int(c), int(m)) for c, m in template)
    if key not in _NC_CACHE:
        _NC_CACHE[key] = _build_bass(key)
    return _NC_CACHE[key]


def _plan(lens):
    """Balance batches across cores by valid-chunk count.

    Returns (assign, template): assign[c][j] = original batch index handled
    by core c, slot j; template[j] = (chunks, first_mask_sub) compiled for
    slot j. Slots descend in size so the drain tail is short.
    """
    lens = np.maximum(np.asarray(lens), 1)
    chunks = np.minimum(np.ceil(lens / CHUNK).astype(int), NCH)
    order = np.argsort(-chunks, kind="stable")  # descending need
    # pure descending slot order (measured best): the DVE stage-B stream
    # builds a backlog from the big early slots and runs gap-free; smallest
    # slot last keeps the drain tail short.
    slot_ranks = list(range(BL))
    assign = [
        [int(order[r * N_CORES + c]) for r in slot_ranks] for c in range(N_CORES)
    ]
    template = []
    for r in slot_ranks:
        group = [int(order[r * N_CORES + c]) for c in range(N_CORES)]
        cmax = int(chunks[order[r * N_CORES]])
        min_len = int(min(lens[b] for b in group))
        template.append((cmax, min_len // SUB))
    return assign, tuple(template)


def prepare_in_maps(enc_outputs, lens, hidden_states, W_enc, b_attn, W_hidden, v):
    """Host-side sharding + layout transforms. Returns (in_maps, assign, t)."""
    enc_outputs = np.asarray(enc_outputs, dtype=np.float32)
    lens = np.asarray(lens, dtype=np.int32)
    hidden_states = np.asarray(hidden_states, dtype=np.float32)
    W_enc = np.asarray(W_enc, dtype=np.float32)
    b_attn = np.asarray(b_attn, dtype=np.float32)
    W_hidden = np.asarray(W_hidden, dtype=np.float32)
    v = np.asarray(v, dtype=np.float32)

    assign, template = _plan(lens)

    # (L, B, D) -> (B, D, L) contiguous fp16 (halves the HBM traffic)
    encT = np.ascontiguousarray(enc_outputs.transpose(1, 2, 0).astype(np.float16))
    w_enc_r = W_enc.astype(np.float16)
    vrep = np.ascontiguousarray(np.repeat(v.astype(np.float16)[:, None], 128, axis=1))
    ones1 = np.ones((1, 128), dtype=ml_dtypes.float8_e5m2)
    ident = np.eye(128, dtype=np.float32)
    b_attn_c = np.ascontiguousarray(b_attn[:, None])

    # length mask rows: 0 where l < lens[b], -30000 where l >= lens[b]
    li = np.arange(L, dtype=np.int32)[None, :]
    mask_full = np.where(li < lens[:, None], 0.0, -30000.0).astype(
        ml_dtypes.float8_e5m2
    )  # (B, L)

    # per-partition tanh bias, computed on host: b_attn + (hidden@W_hidden).T
    hplus_all = (hidden_states @ W_hidden).T + b_attn_c  # (A, B)

    in_maps = []
    for c in range(N_CORES):
        bs = assign[c]
        # packed enc: per slot, [128, DC*(n_l+PADC)] with zero pads
        secs = []
        for j, (C, _) in enumerate(template):
            n_l = C * CHUNK
            eb = encT[bs[j]].reshape(DC, 128, L)[:, :, :n_l]  # (DC,128,n_l)
            ebp = np.zeros((DC, 128, n_l + PADC), dtype=np.float16)
            ebp[:, :, :n_l] = eb
            secs.append(ebp.transpose(1, 0, 2).reshape(128, DC * (n_l + PADC)))
        encP = np.ascontiguousarray(np.concatenate(secs, axis=1))
        in_maps.append(
            {
                "encP": encP,
                "msk": np.ascontiguousarray(mask_full[bs]).reshape(1, BL * L),
                "hplus": np.ascontiguousarray(hplus_all[:, bs]),
                "w_enc": w_enc_r,
                "vrep": vrep,
                "ones1": ones1,
                "ident": ident,
            }
        )
    return in_maps, assign, template


def _run(inputs_np, trace=False):
    in_maps, assign, template = prepare_in_maps(**inputs_np)
    nc = _get_nc(template)
    res = run_bass_kernel_spmd(
        nc, in_maps, core_ids=list(range(N_CORES)), trace=trace
    )
    out = np.empty((B, D), dtype=np.float32)
    for c in range(N_CORES):
        rows = res.results[c]["out"].reshape(BL, D)
        s = res.results[c]["outS"].reshape(BL)
        for j in range(BL):
            out[assign[c][j]] = rows[j] / s[j]
    return out, res


def kernel(enc_outputs, lens, hidden_states, W_enc, b_attn, W_hidden, v, **kwargs):
    out, _ = _run(
        dict(
            enc_outputs=enc_outputs, lens=lens, hidden_states=hidden_states,
            W_enc=W_enc, b_attn=b_attn, W_hidden=W_hidden, v=v,
        )
    )
    return out


def kernel_traced(enc_outputs, lens, hidden_states, W_enc, b_attn, W_hidden, v):
    """Like kernel() but returns (output, BassKernelResults with trace)."""
    return _run(
        dict(
            enc_outputs=enc_outputs, lens=lens, hidden_states=hidden_states,
            W_enc=W_enc, b_attn=b_attn, W_hidden=W_hidden, v=v,
        ),
        trace=True,
    )
